# revision 4
# baseline (speedup 1.0000x reference)
"""BitNet-style quantized 4-layer MLP on 8 Trainium2 NeuronCores — v2.

Data-parallel over the batch (8192 -> 1024 rows/core). Weights are
ternary-quantized ON HOST (exact BitNet per-tensor quantization in numpy),
shipped as int8 {-1,0,1} per-core row-shards (5.2 MB/core) and re-assembled
on device by four int8 AllGathers (Shared outputs) that overlap layer-1
compute. x is int8-quantized per-row on host and shipped pre-transposed
(feature-major), so the device does no transposes at all:

 - Layers 1-3 run weights-stationary: out = W_tile.T @ actsT, producing
   feature-major PSUM tiles [feat128 x batch512] whose layout is directly the
   next layer's k-major input. Epilogue: DVE mult by the per-token dequant
   scale (broadcast tile), ACT tanh with per-feature bias, abs-max tracking.
 - Per-token absmax (a partition-axis reduction in this layout) uses
   gpsimd.partition_all_reduce(absmax), which also broadcasts the result to
   all partitions -- the quantization scale tiles need no further broadcast.
 - Activation quantization uses the magic-constant (1.5*2^23) RNE round.
   h is staged to DRAM in f32 between the matmul pass and the quant pass
   (absmax must finalize first); the quant pass of batch-chunk 0 overlaps
   the matmul pass of batch-chunk 1.
 - Layer 4 runs acts-stationary (acts k-major tiles as lhsT), producing
   batch-major output directly -- no final transpose. Per-batch dequant
   scales become per-partition [128,1] columns via a tiny strided DMA.
 - All matmul operands are small integers (acts in [-127,127], weights in
   {-1,0,1}) in bf16, so PSUM f32 accumulation is exact.

Per-core roofline: 85.9 GFLOP @ 78.6 TF/s bf16 ~= 1.09 ms of PE time; all
DMA/DVE/ACT/collective work is sized to hide under it.
"""

import sys

if "/opt/trn_rl_repo" not in sys.path:
    sys.path.insert(0, "/opt/trn_rl_repo")

import numpy as np
from contextlib import ExitStack

import concourse.bass as bass
import concourse.bacc as bacc
import concourse.tile as tile
import concourse.mybir as mybir
from concourse import bass_isa

F32 = mybir.dt.float32
F16 = mybir.dt.float16
BF16 = mybir.dt.bfloat16
I8 = mybir.dt.int8
ALU = mybir.AluOpType
AF = mybir.ActivationFunctionType
AX = mybir.AxisListType
ROP = bass_isa.ReduceOp

MAGIC = 12582912.0  # 1.5 * 2^23: x + MAGIC - MAGIC == RNE-round(x) for |x| < 2^21
EPS = 1e-5
N_CORES = 8
AUXW = 1024  # aux tensor row width

FULL_CFG = dict(B_CORE=1024, D_IN=1024, H=4096, D_OUT=1024)


def _aux_rows(dims, B_CORE, W=AUXW):
    """Row offsets in the aux tensor: b4 row, ds1, mu row, bias-col block.

    The bias-col block holds biases of layers 1-3 pre-transposed into
    [128, NBCOL] column layout (value[p, col] = b_l[f*128+p]) so the device
    loads it with one contiguous-run DMA instead of a 4-byte-element
    scatter."""
    offs = [0, 0, 0, 0]  # b1-b3 ride in the bias-col block; offs[3] = b4 row
    r = 1
    ds_row = r
    r += (B_CORE + W - 1) // W
    mu_row = r
    r += 1
    nbcol = sum(dims[l + 1] // 128 for l in range(3))
    bc_row = r
    r += (128 * nbcol + W - 1) // W
    return offs, ds_row, mu_row, bc_row, nbcol, r


def build_model(nc, B_CORE, D_IN, H, D_OUT, n_cores=N_CORES, repeats=1,
                no_cc=False, fake_par=False):
    dims = [D_IN, H, H, H, D_OUT]
    NL = 4
    NBC = max(1, B_CORE // 512)          # batch chunks per core
    BCW = B_CORE // NBC                  # batch chunk width (<=512)
    assert BCW <= 512 and B_CORE % 128 == 0
    assert all(d % 128 == 0 for d in dims)
    for l in range(3):
        assert (dims[l + 1] // 128) % n_cores == 0  # f'-tile sharding
    assert (dims[3] // 128) % n_cores == 0          # L4 kt sharding

    SZ = [dims[l] * dims[l + 1] // n_cores for l in range(NL)]
    WOFF = [sum(SZ[:l]) for l in range(NL)]
    WTOT = sum(SZ)
    b_offs, ds_row, mu_row, bc_row, nbcol, aux_R = _aux_rows(dims, B_CORE)

    XQB = D_IN * B_CORE
    blob_d = nc.dram_tensor("blob", [XQB + WTOT], I8, kind="ExternalInput")
    xq_d = blob_d[0:XQB].rearrange("(a b) -> a b", b=B_CORE)
    ws_d = blob_d[XQB:XQB + WTOT]
    aux_d = nc.dram_tensor("aux", [aux_R, AUXW], F32, kind="ExternalInput")
    out_d = nc.dram_tensor("out", [B_CORE, D_OUT], BF16, kind="ExternalOutput")

    with ExitStack() as ctx:
        tc = ctx.enter_context(tile.TileContext(nc))
        sb = ctx.enter_context(tc.tile_pool(name="sb", bufs=1))
        dram = ctx.enter_context(tc.tile_pool(name="dram", bufs=1, space="DRAM"))
        psum = ctx.enter_context(tc.tile_pool(name="ps", bufs=1, space="PSUM"))

        # ---------- constants / aux ----------
        negmagic = sb.tile([128, 1], F32, name="negmagic")
        nc.vector.memset(negmagic[:], -MAGIC)
        ones_row = sb.tile([1, 128], F32, name="ones_row")
        nc.vector.memset(ones_row[:], 1.0)

        # per-feature bias columns for layers 1-3, pre-transposed on host:
        # one contiguous-run DMA into [128, nbcol], then sliced per layer.
        bias_blk = sb.tile([128, nbcol], F32, name="bias_blk")
        bq = (128 * nbcol + AUXW - 1) // AUXW
        nc.sync.dma_start(
            bias_blk[:],
            aux_d[bc_row:bc_row + bq, :].rearrange("a b -> (a b)")
            [0:128 * nbcol].rearrange("(p j) -> p j", p=128))
        bias_cols = []
        off = 0
        for l in range(3):
            NF = dims[l + 1] // 128
            bias_cols.append(bias_blk[:, off:off + NF])
            off += NF
        # L4 bias as a broadcast row tile [128, D_OUT]
        b4row = sb.tile([1, D_OUT], F32, name="b4row")
        nc.sync.dma_start(b4row[:], aux_d[b_offs[3]:b_offs[3] + 1, 0:D_OUT])
        bias4 = sb.tile([128, D_OUT], F32, name="bias4")
        nc.gpsimd.partition_broadcast(bias4[:], b4row[:])
        # ds1 row (per-batch dequant scale incl. mu1), mu constants
        dsxrow = sb.tile([1, B_CORE], F32, name="dsxrow")
        nc.sync.dma_start(dsxrow[:], aux_d[ds_row:ds_row + 1, 0:B_CORE])
        murow = sb.tile([1, 4], F32, name="murow")
        nc.sync.dma_start(murow[:], aux_d[mu_row:mu_row + 1, 0:4])
        mub = sb.tile([128, 4], F32, name="mub")
        nc.gpsimd.partition_broadcast(mub[:], murow[:])

        # L1 scale broadcast tiles — emitted BEFORE the collective triggers so
        # the gpsimd queue isn't blocked behind AllGather completion waits.
        scale_pre = {}
        for bc in range(NBC):
            s = sb.tile([128, BCW], F32, tag="scl", bufs=2 * NBC,
                        name=f"sclpre_{bc}")
            nc.gpsimd.partition_broadcast(
                s[:], dsxrow[0:1, bc * BCW:(bc + 1) * BCW])
            scale_pre[bc] = s

        # ---------- weight shard staging + AllGathers ----------
        ROW = 4096
        assert all(s % ROW == 0 for s in SZ)
        ag_in = [dram.tile([SZ[l] // ROW, ROW], I8, name=f"agin{l}")
                 for l in range(NL)]
        ag_out = [dram.tile([n_cores, SZ[l]], I8, addr_space="Shared",
                            name=f"agout{l}") for l in range(NL)]
        for l in range(NL):
            eng = nc.sync if l % 2 == 0 else nc.scalar
            eng.dma_start(
                ag_in[l][:], ws_d[WOFF[l]:WOFF[l] + SZ[l]]
                .rearrange("(a b) -> a b", b=ROW))
            if no_cc:
                # n_cores==1: exact. n_cores>1: timing-only bisection build
                # (only shard 0 of ag_out is filled; numerics are garbage).
                nc.sync.dma_start(
                    ag_out[l][0:1, :],
                    ag_in[l].rearrange("a b -> (a b)")
                    .rearrange("(q s) -> q s", q=1))
            else:
                nc.gpsimd.collective_compute(
                    "AllGather", ALU.bypass,
                    replica_groups=[list(range(n_cores))],
                    ins=[ag_in[l].opt()], outs=[ag_out[l].opt()])

        def par_allreduce(out_ap, in_ap, tagp=""):
            if fake_par:  # timing-only probe: numerics are garbage
                nc.vector.memset(out_ap, 0.01)
            else:
                nc.gpsimd.partition_all_reduce(out_ap, in_ap, 128, ROP.absmax)

        def par_broadcast(out_ap, in_ap):
            if fake_par:
                nc.vector.memset(out_ap, 0.01)
            else:
                nc.gpsimd.partition_broadcast(out_ap, in_ap)

        def ag_flat(l):
            return ag_out[l].rearrange("a b -> (a b)")

        def w_chunk(l, f, k0, kn):
            """[128, kn*128] int8 slice of layer-l f'-tile (layers 0..2).

            Full-blob layout per f'-tile: [p, kt, c] p-major."""
            KT = dims[l] // 128
            base = f * (dims[l] * 128)
            ap = ag_flat(l)[base:base + 128 * KT * 128] \
                .rearrange("(p x) -> p x", p=128)
            return ap[:, k0 * 128:(k0 + kn) * 128]

        def w4_chunk(kt):
            """[128, D_OUT] int8 k-tile of layer 4 (row-major W^T)."""
            base = kt * 128 * D_OUT
            return ag_flat(3)[base:base + 128 * D_OUT] \
                .rearrange("(p c) -> p c", p=128)

        KTs = [dims[l] // 128 for l in range(NL)]
        for _rep in range(repeats):
            if _rep == 0:
                scale_cur = dict(scale_pre)
            else:
                scale_cur = {}
                for bc in range(NBC):
                    s = sb.tile([128, BCW], F32, tag="scl", bufs=2 * NBC,
                                name=f"scl0_{bc}")
                    par_broadcast(
                        s[:], dsxrow[0:1, bc * BCW:(bc + 1) * BCW])
                    scale_cur[bc] = s
            # ---------- x load + cast ----------
            xq = {}  # (l, kt, bc) -> bf16 [128, BCW]
            xq_tags = [f"xqA", f"xqB"]  # ping-pong across layers
            xq_bufs = [max(KTs[0], KTs[2]) * NBC, max(KTs[1], KTs[3]) * NBC]
            for kt in range(KTs[0]):
                xk = sb.tile([128, B_CORE], I8, tag="xk", bufs=2,
                             name=f"xk{kt}")
                nc.sync.dma_start(xk[:], xq_d[kt * 128:(kt + 1) * 128, :])
                for bc in range(NBC):
                    t = sb.tile([128, BCW], BF16, tag=xq_tags[0],
                                bufs=xq_bufs[0], name=f"xq0_{kt}_{bc}")
                    nc.vector.tensor_copy(t[:], xk[:, bc * BCW:(bc + 1) * BCW])
                    xq[(0, kt, bc)] = t

            # ---------- layers 1-3 (weights stationary, feature-major) ------
            for l in range(3):
                KT = dims[l] // 128
                NF = dims[l + 1] // 128
                KC = min(KT, 8)
                NKC = KT // KC
                scale_next = {}
                for bc in range(NBC):
                    runmax = sb.tile([128, BCW], F32, tag="rmax", bufs=2,
                                     name=f"rm{l}_{bc}")
                    nc.vector.memset(runmax[:], 0.0)
                    runmin = sb.tile([128, BCW], F32, tag="rmin", bufs=2,
                                     name=f"rn{l}_{bc}")
                    nc.vector.memset(runmin[:], 0.0)
                    hdr = dram.tile([NF * 128, BCW], F32, tag="hdr", bufs=2,
                                    name=f"hdr{l}_{bc}")
                    for f in range(NF):
                        ps = psum.tile([128, BCW], F32, tag="mm", bufs=8,
                                       name=f"ps{l}_{bc}_{f}")
                        for kc in range(NKC):
                            wi = sb.tile([128, KC * 128], I8, tag="wi", bufs=3,
                                         name=f"wi{l}_{bc}_{f}_{kc}")
                            nc.scalar.dma_start(wi[:], w_chunk(l, f, kc * KC, KC))
                            wb = sb.tile([128, KC * 128], BF16, tag="wb",
                                         bufs=3, name=f"wb{l}_{bc}_{f}_{kc}")
                            nc.vector.tensor_copy(wb[:], wi[:])
                            for k in range(KC):
                                kt = kc * KC + k
                                nc.tensor.matmul(
                                    ps[:], wb[:, k * 128:(k + 1) * 128],
                                    xq[(l, kt, bc)][:],
                                    start=(kt == 0), stop=(kt == KT - 1))
                        tt = sb.tile([128, BCW], F32, tag="ept", bufs=2,
                                     name=f"tt{l}_{bc}_{f}")
                        nc.vector.tensor_tensor(tt[:], ps[:],
                                                scale_cur[bc][:], ALU.mult)
                        h = sb.tile([128, BCW], F32, tag="h", bufs=3,
                                    name=f"h{l}_{bc}_{f}")
                        nc.scalar.activation(h[:], tt[:], AF.Tanh,
                                             bias=bias_cols[l][:, f:f + 1])
                        nc.vector.tensor_tensor(runmax[:], runmax[:], h[:],
                                                ALU.max)
                        nc.vector.tensor_tensor(runmin[:], runmin[:], h[:],
                                                ALU.min)
                        nc.sync.dma_start(hdr[f * 128:(f + 1) * 128, :], h[:])
                    # absmax -> scales (broadcast across partitions already)
                    comb = sb.tile([128, BCW], F32, tag="comb", bufs=1,
                                   name=f"cb{l}_{bc}")
                    nc.vector.scalar_tensor_tensor(comb[:], runmin[:], -1.0,
                                                   runmax[:], ALU.mult, ALU.max)
                    amax = sb.tile([128, BCW], F32, tag="amax", bufs=1,
                                   name=f"am{l}_{bc}")
                    par_allreduce(amax[:], comb[:], tagp=f"{l}_{bc}")
                    z = sb.tile([128, BCW], F32, tag="zt", bufs=1,
                                name=f"z{l}_{bc}")
                    nc.vector.tensor_scalar(z[:], amax[:], EPS, None, ALU.max)
                    rz = sb.tile([128, BCW], F32, tag="rzt", bufs=1,
                                 name=f"rz{l}_{bc}")
                    nc.vector.reciprocal(rz[:], z[:])
                    qs = sb.tile([128, BCW], F32, tag="qst", bufs=2,
                                 name=f"qs{l}_{bc}")
                    nc.vector.tensor_scalar(qs[:], rz[:], 127.0, None,
                                            ALU.mult)
                    sn = sb.tile([128, BCW], F32, tag="scl", bufs=2 * NBC,
                                 name=f"scl{l+1}_{bc}")
                    nc.vector.tensor_scalar(sn[:], z[:], mub[:, l:l + 1],
                                            None, ALU.mult)
                    scale_next[bc] = sn
                    # quant pass: read h back, round to int-valued bf16
                    for f in range(NF):
                        hb = sb.tile([128, BCW], F32, tag="hb", bufs=2,
                                     name=f"hb{l}_{bc}_{f}")
                        nc.sync.dma_start(hb[:], hdr[f * 128:(f + 1) * 128, :])
                        nc.vector.tensor_tensor(hb[:], hb[:], qs[:], ALU.mult)
                        nc.vector.tensor_scalar(hb[:], hb[:], MAGIC, None,
                                                ALU.add)
                        xt = sb.tile([128, BCW], BF16, tag=xq_tags[(l + 1) % 2],
                                     bufs=xq_bufs[(l + 1) % 2],
                                     name=f"xq{l+1}_{f}_{bc}")
                        nc.vector.tensor_scalar(xt[:], hb[:], -MAGIC, None,
                                                ALU.add)
                        xq[(l + 1, f, bc)] = xt
                scale_cur = scale_next

            # ---------- layer 4 (acts stationary, batch-major out) ----------
            KT4 = KTs[3]
            M_T = B_CORE // 128
            MPB = BCW // 128            # m-tiles per batch chunk
            NW4 = min(512, D_OUT)
            NC4 = D_OUT // NW4
            # per-batch dequant scale columns [128, M_T]: bounce one row of the
            # (partition-broadcast) scale tile through DRAM, scatter back with
            # a partition-creating rearrange (only legal on the DRAM side).
            ds4 = sb.tile([128, M_T], F32, name="ds4", tag="ds4", bufs=2)
            for bc in range(NBC):
                dsd = dram.tile([1, BCW], F32, tag="dsd", bufs=2,
                                name=f"dsd{bc}")
                nc.sync.dma_start(dsd[:], scale_cur[bc][0:1, :])
                nc.sync.dma_start(
                    ds4[:, bc * MPB:(bc + 1) * MPB],
                    dsd.rearrange("q (m p) -> (q p) m", p=128))
            MG = min(4, M_T)
            for mg0 in range(0, M_T, MG):
                pss = {}
                for kt in range(KT4):
                    w4i = sb.tile([128, D_OUT], I8, tag="w4i", bufs=3,
                                  name=f"w4i_{mg0}_{kt}")
                    nc.scalar.dma_start(w4i[:], w4_chunk(kt))
                    w4b = sb.tile([128, D_OUT], BF16, tag="w4b", bufs=3,
                                  name=f"w4b_{mg0}_{kt}")
                    nc.vector.tensor_copy(w4b[:], w4i[:])
                    for mi in range(MG):
                        m = mg0 + mi
                        bc = m // MPB
                        mc = (m % MPB) * 128
                        lhsT = xq[(3, kt, bc)][:, mc:mc + 128]
                        for n in range(NC4):
                            if kt == 0:
                                pss[(mi, n)] = psum.tile(
                                    [128, NW4], F32, tag="mm", bufs=8,
                                    name=f"ps4_{mg0}_{mi}_{n}")
                            nc.tensor.matmul(
                                pss[(mi, n)][:], lhsT,
                                w4b[:, n * NW4:(n + 1) * NW4],
                                start=(kt == 0), stop=(kt == KT4 - 1))
                for mi in range(MG):
                    m = mg0 + mi
                    for n in range(NC4):
                        ot = sb.tile([128, NW4], BF16, tag="ot", bufs=2,
                                     name=f"ot{m}_{n}")
                        nc.vector.scalar_tensor_tensor(
                            ot[:], pss[(mi, n)][:], ds4[:, m:m + 1],
                            bias4[:, n * NW4:(n + 1) * NW4],
                            ALU.mult, ALU.add)
                        nc.sync.dma_start(
                            out_d[m * 128:(m + 1) * 128,
                                  n * NW4:(n + 1) * NW4], ot[:])

    return dict(blob=blob_d, aux=aux_d, out=out_d)


# ----------------------------------------------------------------------------
# Host-side quantization + input packing
# ----------------------------------------------------------------------------

def _quant_weight(w):
    """Exact BitNet per-tensor ternary quantization. Returns (t int8, mu)."""
    w = np.asarray(w, dtype=np.float32)
    mu = np.float32(max(np.abs(w.astype(np.float64)).mean(), EPS))
    scale = np.float32(1.0) / mu
    t = np.clip(np.rint(w * scale), -1.0, 1.0).astype(np.int8)
    return t, mu


def _pack_weight_A(t):
    """Layers 1-3 blob: per f'-tile [p, kt, c] p-major, f'-tiles concatenated."""
    T = t.T  # [in, out]
    din, dout = T.shape
    KT, NF = din // 128, dout // 128
    blk = T.reshape(KT, 128, NF, 128)
    return np.ascontiguousarray(blk.transpose(2, 1, 0, 3)).ravel()


def make_in_maps(inputs, cfg=None, n_cores=N_CORES):
    cfg = cfg or FULL_CFG
    B_CORE, D_IN = cfg["B_CORE"], cfg["D_IN"]
    H, D_OUT = cfg["H"], cfg["D_OUT"]
    dims = [D_IN, H, H, H, D_OUT]
    x = np.asarray(inputs["x"], dtype=np.float32)

    ts, mus = [], []
    for l in range(4):
        t, mu = _quant_weight(inputs[f"w{l+1}"])
        ts.append(t)
        mus.append(mu)
    flats = [_pack_weight_A(ts[l]) for l in range(3)]
    flats.append(np.ascontiguousarray(ts[3].T).ravel())

    SZ = [dims[l] * dims[l + 1] // n_cores for l in range(4)]
    b_offs, ds_row, mu_row, bc_row, nbcol, aux_R = _aux_rows(dims, B_CORE)

    aux_base = np.zeros((aux_R, AUXW), np.float32)
    b4 = np.asarray(inputs["b4"], dtype=np.float32).ravel()
    aux_base[b_offs[3], :b4.size] = b4
    blk = np.concatenate(
        [np.asarray(inputs[f"b{l+1}"], np.float32).reshape(-1, 128).T
         for l in range(3)], axis=1)          # [128, nbcol]
    aux_base[bc_row:].reshape(-1)[:128 * nbcol] = blk.ravel()
    aux_base[mu_row, 0:3] = [mus[1] / np.float32(127.0),
                             mus[2] / np.float32(127.0),
                             mus[3] / np.float32(127.0)]

    in_maps = []
    for c in range(n_cores):
        xs = x[c * B_CORE:(c + 1) * B_CORE]
        amax = np.abs(xs).max(axis=1)
        z = np.maximum(amax, np.float32(EPS)).astype(np.float32)
        qsc = np.float32(127.0) / z
        xq = np.rint(xs * qsc[:, None]).astype(np.int8)
        ds1 = (z / np.float32(127.0)) * mus[0]
        aux = aux_base.copy()
        aux[ds_row:].reshape(-1)[:B_CORE] = ds1
        ws = np.concatenate(
            [flats[l][c * SZ[l]:(c + 1) * SZ[l]] for l in range(4)])
        in_maps.append({
            "blob": np.concatenate([np.ascontiguousarray(xq.T).ravel(), ws]),
            "aux": aux,
        })
    return in_maps


# ----------------------------------------------------------------------------
# Host wrapper (compile/run/bench) — same mechanics as the v1 baseline
# ----------------------------------------------------------------------------

_CACHE = {}


def _compiled(cfg=None, n_cores=N_CORES, no_cc=False, repeats=1,
              fake_par=False):
    cfg = cfg or FULL_CFG
    key = (tuple(sorted(cfg.items())), n_cores, no_cc, repeats, fake_par)
    if key not in _CACHE:
        nc = bacc.Bacc("TRN2", target_bir_lowering=False, debug=False,
                       enable_asserts=True, num_devices=n_cores)
        build_model(nc, n_cores=n_cores, no_cc=no_cc, repeats=repeats,
                    fake_par=fake_par, **cfg)
        nc.compile()
        _CACHE[key] = nc
    return _CACHE[key]


def run(inputs, trace=False, cfg=None):
    from concourse.bass_utils import run_bass_kernel_spmd
    cfg = cfg or FULL_CFG
    nc = _compiled(cfg)
    in_maps = make_in_maps(inputs, cfg)
    last_err = None
    for _attempt in range(3):
        try:
            res = run_bass_kernel_spmd(nc, in_maps,
                                       core_ids=list(range(N_CORES)),
                                       trace=trace)
            break
        except Exception as e:  # transient NRT/axon failures
            last_err = e
            import time
            time.sleep(5.0)
    else:
        raise last_err
    out = np.concatenate([np.asarray(res.results[k]["out"])
                          for k in range(N_CORES)], axis=0)
    return out.astype(np.float32), res.exec_time_ns


def kernel(**inputs):
    out, _ = run(inputs)
    return out


def _make_pjrt_callable(nc, in_maps, n_cores=N_CORES):
    """Build a (jitted_fn, device_args, out_names, out_avals) for repeated
    execution of nc's NEFF on n_cores with device-resident inputs."""
    import jax
    import concourse.mybir as mb
    from jax.sharding import Mesh, PartitionSpec
    from jax.experimental.shard_map import shard_map
    from concourse.bass2jax import (_bass_exec_p, partition_id_tensor,
                                    install_neuronx_cc_hook)

    install_neuronx_cc_hook()
    partition_name = nc.partition_id_tensor.name if nc.partition_id_tensor else None
    in_names, out_names, out_avals, zero_outs = [], [], [], []
    for alloc in nc.m.functions[0].allocations:
        if not isinstance(alloc, mb.MemoryLocationSet):
            continue
        name = alloc.memorylocations[0].name
        if alloc.kind == "ExternalInput":
            if name != partition_name:
                in_names.append(name)
        elif alloc.kind == "ExternalOutput":
            out_names.append(name)
            shape = tuple(alloc.tensor_shape)
            dtype = mb.dt.np(alloc.dtype)
            out_avals.append(jax.core.ShapedArray(shape, dtype))
            zero_outs.append(np.zeros(shape, dtype))
    n_params = len(in_names)
    all_in_names = in_names + out_names
    if partition_name is not None:
        all_in_names.append(partition_name)

    def _body(*args):
        pid = [partition_id_tensor()] if partition_name is not None else []
        outs = _bass_exec_p.bind(
            *args, *pid,
            out_avals=tuple(out_avals),
            in_names=tuple(all_in_names),
            out_names=tuple(out_names),
            lowering_input_output_aliases=(),
            sim_require_finite=True,
            sim_require_nnan=True,
            nc=nc,
        )
        return tuple(outs)

    devices = jax.devices()[:n_cores]
    mesh = Mesh(np.asarray(devices), ("core",))
    n_outs = len(out_names)
    fn = jax.jit(
        shard_map(_body, mesh=mesh,
                  in_specs=(PartitionSpec("core"),) * (n_params + n_outs),
                  out_specs=(PartitionSpec("core"),) * n_outs,
                  check_rep=False),
        keep_unused=True,
    )
    per_core = [[np.asarray(in_maps[c][n]) for n in in_names]
                for c in range(n_cores)]
    concat_in = [np.concatenate([per_core[c][i] for c in range(n_cores)], axis=0)
                 for i in range(n_params)]
    concat_zeros = [np.zeros((n_cores * z.shape[0], *z.shape[1:]), z.dtype)
                    for z in zero_outs]
    args = [jax.device_put(a) for a in concat_in + concat_zeros]
    return fn, args, out_names, out_avals


def _calib_nc():
    """Tiny 8-core kernel used to measure per-call dispatch overhead."""
    nc = bacc.Bacc("TRN2", target_bir_lowering=False, debug=False,
                   enable_asserts=True, num_devices=N_CORES)
    xi = nc.dram_tensor("xi", [1, 128], F32, kind="ExternalInput")
    xo = nc.dram_tensor("xo", [1, 128], F32, kind="ExternalOutput")
    with ExitStack() as ctx:
        tc = ctx.enter_context(tile.TileContext(nc))
        sb = ctx.enter_context(tc.tile_pool(name="sb", bufs=1))
        t = sb.tile([1, 128], F32, name="t")
        nc.sync.dma_start(t[:], xi[:])
        nc.sync.dma_start(xo[:], t[:])
    nc.compile()
    return nc


def bench(inputs, iters=10, cfg=None, repeats=1):
    """Returns (out, est_exec_seconds): median per-call wall time on
    device-resident inputs, minus per-call dispatch overhead measured with a
    trivial kernel. With repeats>1 the model body runs `repeats` times per
    call and the estimate is divided accordingly (weight AllGathers run
    once, so this slightly under-reports their cost)."""
    import time
    import jax

    cfg = cfg or FULL_CFG
    nc = _compiled(cfg, repeats=repeats)
    in_maps = make_in_maps(inputs, cfg)
    fn, args, out_names, _ = _make_pjrt_callable(nc, in_maps)
    out_arrs = jax.block_until_ready(fn(*args))   # compile + warm
    times = []
    for _ in range(iters):
        t0 = time.perf_counter()
        jax.block_until_ready(fn(*args))
        times.append(time.perf_counter() - t0)
    big = float(np.median(times))

    cnc = _calib_nc()
    cmaps = [{"xi": np.zeros((1, 128), np.float32)} for _ in range(N_CORES)]
    cfn, cargs, _, _ = _make_pjrt_callable(cnc, cmaps)
    jax.block_until_ready(cfn(*cargs))
    ctimes = []
    for _ in range(iters):
        t0 = time.perf_counter()
        jax.block_until_ready(cfn(*cargs))
        ctimes.append(time.perf_counter() - t0)
    small = float(np.median(ctimes))

    oi = out_names.index("out")
    B_CORE = cfg["B_CORE"]
    out = np.asarray(out_arrs[oi]).reshape(N_CORES * B_CORE, -1)
    est = max(big - small, 0.0) / repeats
    print(f"[bench] per-call wall: {big*1e3:.3f} ms; dispatch overhead: "
          f"{small*1e3:.3f} ms; est exec: {est*1e3:.3f} ms (repeats={repeats})")
    return out.astype(np.float32), est
